# revision 6
# baseline (speedup 1.0000x reference)
"""Two-layer GCN encoder (GCNConv x2 -> mu/logvar heads) on 8 TRN2 NeuronCores.

Strategy (self-contained, full inputs in / full outputs out):
  - Fold the symmetric normalization dinv[src]*dinv[dst] into the stored
    per-layer feature tables: z1' = dinv * (x @ W1), z2' = dinv * (h1 @ W2)
    (row scales), so message aggregation is a plain segment-sum.
  - Every core computes the full z1' table (replicated: x^T is shipped to
    every core as bf16, so no cross-core exchange is needed for layer 1).
  - Edges are partitioned by destination: core c owns dst nodes
    [c*6250, (c+1)*6250), as 49 blocks of 128 dst nodes. Edges are sorted by
    (src-half, dst) on the host; per (block, half) groups are padded to a
    core-independent tile count (SPMD: one program, 8 cores).
  - Messages are fetched with dma_gather (bf16 rows of 256B; int16 indices,
    hence the two 25000-row source halves) and scattered into PSUM with a
    one-hot matmul: h^T += Msg^T @ Sel, Sel[e, d] = (dst_local[e] == d),
    generated on the DVE from iota/is_equal.
  - z2' shards are exchanged with one AllGather (padded to 128 features so
    layer-2 gather rows stay 256B).
  - mu/logvar = dinv * (S2 @ W) (+ folded biases via a rank-1 matmul when
    biases are nonzero; they are zero for this module).
"""

import os

import ml_dtypes
import numpy as np

import concourse.bacc as bacc
import concourse.bass as bass
import concourse.mybir as mybir
import concourse.tile as tile
from concourse import library_config
from concourse.bass_utils import run_bass_kernel_spmd

# ---- problem constants (hardcoded per harness contract) ----
N = 50000
IN_D, HID1, HID2, OUT_D = 256, 128, 64, 32
NC_CORES = 8
NSH = N // NC_CORES  # 6250 dst nodes per core
NBLK = (NSH + 127) // 128  # 49 dst blocks per core
HALF = 25000  # int16 gather index limit => two source halves
P = 128
CHUNK_BLOCKS = 7  # dst blocks per gather chunk (49 = 7*7)

BF16 = ml_dtypes.bfloat16

_tile_patched = False


def _patch_tile_drain():
    """walrus in this env rejects >~2 sem waits on one instruction; Tile's
    kernel-tail drain aggregates one wait per live semaphore. Move the excess
    onto dedicated single-wait SP nops that precede the drain."""
    global _tile_patched
    if _tile_patched:
        return
    _tile_patched = True
    _orig = tile.TileContext._drain_and_barrier

    def _patched(self, tick_clock, wait_clock):
        nc = self.nc
        nops = [nc.sync.nop(nofuse=True, hint=f"dw_{i}").ins for i in range(64)]
        _orig(self, tick_clock, wait_clock)
        ni = 0
        for inst in nc.cur_bb.bb.instructions:
            if "Drain" not in type(inst).__name__:
                continue
            ow = inst.sync_info.on_wait if inst.sync_info else []
            if len(ow) > 1:
                waits = list(ow)
                for w in waits[:-1]:
                    nops[ni].sync_info = mybir.SyncInfo(on_wait=[w], on_update=[])
                    ni += 1
                inst.sync_info.on_wait[:] = waits[-1:]

    tile.TileContext._drain_and_barrier = _patched


def _prep(x, edge_index, W1, b1, W2, b2, W_mu, b_mu, W_lv, b_lv):
    """Host-side graph partitioning + input staging. Returns (in_maps, plan)."""
    src = np.asarray(edge_index[0], dtype=np.int64)
    dst = np.asarray(edge_index[1], dtype=np.int64)
    loop = np.arange(N, dtype=np.int64)
    src_a = np.concatenate([src, loop])
    dst_a = np.concatenate([dst, loop])

    deg = np.bincount(dst_a, minlength=N).astype(np.float64)
    dinv = deg**-0.5
    invdeg = 1.0 / deg
    sqrtdeg = deg**0.5

    # sort edges by (src-half, dst) so each (dst-block, half) group is a
    # contiguous run
    half = (src_a >= HALF).astype(np.int64)
    key = half * N + dst_a
    order = np.argsort(key, kind="stable")
    s_sorted = src_a[order]
    d_sorted = dst_a[order]
    bnd = np.searchsorted(key[order], np.arange(2 * N + 1))

    # per-(core, block, half) counts -> core-independent tile counts
    T = [[0, 0] for _ in range(NBLK)]
    counts = np.zeros((NC_CORES, NBLK, 2), dtype=np.int64)
    for c in range(NC_CORES):
        for b in range(NBLK):
            lo = c * NSH + b * 128
            hi = min(c * NSH + (b + 1) * 128, (c + 1) * NSH)
            for h in range(2):
                cnt = bnd[h * N + hi] - bnd[h * N + lo]
                counts[c, b, h] = cnt
    for b in range(NBLK):
        for h in range(2):
            T[b][h] = max(1, int(-(-counts[:, b, h].max() // 128)))

    TH = [sum(T[b][h] for b in range(NBLK)) for h in range(2)]
    toff = [[0] * NBLK, [0] * NBLK]  # tile offsets per half
    for h in range(2):
        acc = 0
        for b in range(NBLK):
            toff[h][b] = acc
            acc += T[b][h]

    # build per-core padded idx / dstloc streams
    core_data = []
    for c in range(NC_CORES):
        idx_streams = []
        dl_streams = []
        for h in range(2):
            idx = np.zeros(TH[h] * 128, dtype=np.int16)
            dl = np.full(TH[h] * 128, -1.0, dtype=np.float32)
            for b in range(NBLK):
                lo = c * NSH + b * 128
                hi = min(c * NSH + (b + 1) * 128, (c + 1) * NSH)
                e0, e1 = bnd[h * N + lo], bnd[h * N + hi]
                cnt = e1 - e0
                off = toff[h][b] * 128
                idx[off : off + cnt] = (s_sorted[e0:e1] - h * HALF).astype(np.int16)
                dl[off : off + cnt] = (d_sorted[e0:e1] - lo).astype(np.float32)
            # pack: index i -> partition i%16, col i//16, replicated 8x
            packed = np.tile(np.ascontiguousarray(idx.reshape(-1, 16).T), (8, 1))
            idx_streams.append(packed)
            dl_streams.append(
                np.ascontiguousarray(dl.reshape(-1, 128).T).astype(BF16)
            )
        core_data.append((idx_streams, dl_streams))

    # replicated tensors
    xT = np.ascontiguousarray(np.asarray(x, np.float32).T).astype(BF16)  # [256, N]
    NPAD = NBLK * 128 * NC_CORES  # 50176
    NBLK_ALL = -(-N // 128)  # 391
    dinv_all = np.zeros(NBLK_ALL * 128, np.float32)
    dinv_all[:N] = dinv
    dinv_col_all = np.ascontiguousarray(dinv_all.reshape(NBLK_ALL, 128).T)

    w1a = np.asarray(W1[:128], np.float32).astype(BF16)
    w1b = np.asarray(W1[128:], np.float32).astype(BF16)
    w2 = np.asarray(W2, np.float32).astype(BF16)
    wmu = np.asarray(W_mu, np.float32).astype(BF16)
    wlv = np.asarray(W_lv, np.float32).astype(BF16)
    bz2 = (np.asarray(b1, np.float64) @ np.asarray(W2, np.float64)).astype(np.float32)
    bmu = (
        np.asarray(b2, np.float64) @ np.asarray(W_mu, np.float64)
        + np.asarray(b_mu, np.float64)
    ).astype(np.float32)
    blv = (
        np.asarray(b2, np.float64) @ np.asarray(W_lv, np.float64)
        + np.asarray(b_lv, np.float64)
    ).astype(np.float32)
    has_bias = bool(np.any(bz2) or np.any(bmu) or np.any(blv))

    iota_rep = np.tile(np.arange(128, dtype=np.float32), (128, 8)).astype(BF16)

    in_maps = []
    for c in range(NC_CORES):
        (idxA, idxB), (dlA, dlB) = core_data[c]
        own = slice(c * NSH, (c + 1) * NSH)
        sq = np.zeros((1, NBLK * 128), np.float32)
        sq[0, :NSH] = sqrtdeg[own]
        ivc = np.zeros((128, NBLK), np.float32)
        dvc = np.zeros((128, NBLK), np.float32)
        tmp_iv = np.zeros(NBLK * 128, np.float64)
        tmp_dv = np.zeros(NBLK * 128, np.float64)
        tmp_iv[:NSH] = invdeg[own]
        tmp_dv[:NSH] = dinv[own]
        ivc[:, :] = tmp_iv.reshape(NBLK, 128).T
        dvc[:, :] = tmp_dv.reshape(NBLK, 128).T
        in_maps.append(
            {
                "xT": xT,
                "iota": iota_rep,
                "dinv_all": dinv_col_all,
                "idxA": idxA,
                "idxB": idxB,
                "dlA": dlA,
                "dlB": dlB,
                "w1a": w1a,
                "w1b": w1b,
                "w2": w2,
                "wmu": wmu,
                "wlv": wlv,
                "sqrow": sq.astype(BF16),
                "bz2": bz2.reshape(1, -1).astype(BF16),
                "bmu": bmu.reshape(1, -1).astype(BF16),
                "blv": blv.reshape(1, -1).astype(BF16),
                "ivcol": ivc.astype(np.float32),
                "dvcol": dvc.astype(np.float32),
            }
        )

    plan = {
        "T": T,
        "TH": TH,
        "toff": toff,
        "has_bias": has_bias,
    }
    return in_maps, plan


def _build(plan):
    _patch_tile_drain()
    T, TH, toff = plan["T"], plan["TH"], plan["toff"]
    has_bias = plan["has_bias"]
    NBLK_ALL = -(-N // 128)  # 391

    nc = bacc.Bacc("TRN2", num_swdge_queues=4)
    f32, bf16, i16 = mybir.dt.float32, mybir.dt.bfloat16, mybir.dt.int16

    xT_e = nc.dram_tensor("xT", [IN_D, N], bf16, kind="ExternalInput")
    iota_e = nc.dram_tensor("iota", [128, 1024], bf16, kind="ExternalInput")
    dinv_all_e = nc.dram_tensor("dinv_all", [128, NBLK_ALL], f32, kind="ExternalInput")
    idxA_e = nc.dram_tensor("idxA", [128, TH[0] * 8], i16, kind="ExternalInput")
    idxB_e = nc.dram_tensor("idxB", [128, TH[1] * 8], i16, kind="ExternalInput")
    dlA_e = nc.dram_tensor("dlA", [128, TH[0]], bf16, kind="ExternalInput")
    dlB_e = nc.dram_tensor("dlB", [128, TH[1]], bf16, kind="ExternalInput")
    w1a_e = nc.dram_tensor("w1a", [128, HID1], bf16, kind="ExternalInput")
    w1b_e = nc.dram_tensor("w1b", [128, HID1], bf16, kind="ExternalInput")
    w2_e = nc.dram_tensor("w2", [HID1, HID2], bf16, kind="ExternalInput")
    wmu_e = nc.dram_tensor("wmu", [HID2, OUT_D], bf16, kind="ExternalInput")
    wlv_e = nc.dram_tensor("wlv", [HID2, OUT_D], bf16, kind="ExternalInput")
    sqrow_e = nc.dram_tensor("sqrow", [1, NBLK * 128], bf16, kind="ExternalInput")
    bz2_e = nc.dram_tensor("bz2", [1, HID2], bf16, kind="ExternalInput")
    bmu_e = nc.dram_tensor("bmu", [1, OUT_D], bf16, kind="ExternalInput")
    blv_e = nc.dram_tensor("blv", [1, OUT_D], bf16, kind="ExternalInput")
    ivcol_e = nc.dram_tensor("ivcol", [128, NBLK], f32, kind="ExternalInput")
    dvcol_e = nc.dram_tensor("dvcol", [128, NBLK], f32, kind="ExternalInput")

    mu_e = nc.dram_tensor("mu", [NSH, OUT_D], f32, kind="ExternalOutput")
    lv_e = nc.dram_tensor("lv", [NSH, OUT_D], f32, kind="ExternalOutput")

    z1_d = nc.dram_tensor("z1_d", [N, HID1], bf16)
    z2l_d = nc.dram_tensor("z2l_d", [NSH, 128], bf16)
    z2f_d = nc.dram_tensor("z2f_d", [N, 128], bf16, addr_space="Shared")

    core_ids = list(range(NC_CORES))

    # gather chunk plan: groups of CHUNK_BLOCKS dst blocks
    chunks = []
    b0 = 0
    while b0 < NBLK:
        b1 = min(b0 + CHUNK_BLOCKS, NBLK)
        chunks.append((b0, b1))
        b0 = b1

    with tile.TileContext(nc) as tc:
        with (
            tc.tile_pool(name="const", bufs=1) as pc,
            tc.tile_pool(name="xa", bufs=3) as px,
            tc.tile_pool(name="g", bufs=2) as pg,
            tc.tile_pool(name="sel", bufs=6) as psel,
            tc.tile_pool(name="outp", bufs=3) as po,
            tc.tile_pool(name="ps", bufs=2, space="PSUM") as pp,
            tc.tile_pool(name="ps2", bufs=2, space="PSUM") as pp2,
        ):
            nc.gpsimd.load_library(library_config.mlp)

            # ---- resident constants
            iota_t = pc.tile([128, 1024], bf16)
            nc.sync.dma_start(out=iota_t[:], in_=iota_e[:])
            dinv_all_t = pc.tile([128, NBLK_ALL], f32)
            nc.sync.dma_start(out=dinv_all_t[:], in_=dinv_all_e[:])
            idxA_t = pc.tile([128, TH[0] * 8], i16)
            nc.sync.dma_start(out=idxA_t[:], in_=idxA_e[:])
            idxB_t = pc.tile([128, TH[1] * 8], i16)
            nc.sync.dma_start(out=idxB_t[:], in_=idxB_e[:])
            dlA_t = pc.tile([128, TH[0]], bf16)
            nc.sync.dma_start(out=dlA_t[:], in_=dlA_e[:])
            dlB_t = pc.tile([128, TH[1]], bf16)
            nc.sync.dma_start(out=dlB_t[:], in_=dlB_e[:])
            w1a_t = pc.tile([128, HID1], bf16)
            nc.sync.dma_start(out=w1a_t[:], in_=w1a_e[:])
            w1b_t = pc.tile([128, HID1], bf16)
            nc.sync.dma_start(out=w1b_t[:], in_=w1b_e[:])
            w2_t = pc.tile([HID1, HID2], bf16)
            nc.sync.dma_start(out=w2_t[:], in_=w2_e[:])
            wmu_t = pc.tile([HID2, OUT_D], bf16)
            nc.sync.dma_start(out=wmu_t[:], in_=wmu_e[:])
            wlv_t = pc.tile([HID2, OUT_D], bf16)
            nc.sync.dma_start(out=wlv_t[:], in_=wlv_e[:])
            ivcol_t = pc.tile([128, NBLK], f32)
            nc.sync.dma_start(out=ivcol_t[:], in_=ivcol_e[:])
            dvcol_t = pc.tile([128, NBLK], f32)
            nc.sync.dma_start(out=dvcol_t[:], in_=dvcol_e[:])
            if has_bias:
                sqrow_t = pc.tile([1, NBLK * 128], bf16)
                nc.sync.dma_start(out=sqrow_t[:], in_=sqrow_e[:])
                bz2_t = pc.tile([1, HID2], bf16)
                nc.sync.dma_start(out=bz2_t[:], in_=bz2_e[:])
                bmu_t = pc.tile([1, OUT_D], bf16)
                nc.sync.dma_start(out=bmu_t[:], in_=bmu_e[:])
                blv_t = pc.tile([1, OUT_D], bf16)
                nc.sync.dma_start(out=blv_t[:], in_=blv_e[:])

            h1t_all = pc.tile([128, NBLK * 128], bf16)  # h1^T, raw S1
            h2t_all = pc.tile([HID2, NBLK * 128], bf16)  # h2^T, raw S2

            # ---- phase A: z1' = dinv * (x @ W1), all N rows, written to z1_d
            nb_done = 0
            while nb_done < NBLK_ALL:
                nb_cnt = min(8, NBLK_ALL - nb_done)
                c0 = nb_done * 128
                cols = min(nb_cnt * 128, N - c0)
                xa = px.tile([128, 1024], bf16, tag="xa")
                xb = px.tile([128, 1024], bf16, tag="xb")
                nc.sync.dma_start(out=xa[:, :cols], in_=xT_e[0:128, c0 : c0 + cols])
                nc.sync.dma_start(out=xb[:, :cols], in_=xT_e[128:256, c0 : c0 + cols])
                for j in range(nb_cnt):
                    w = min(128, N - (c0 + j * 128))
                    zp = pp.tile([128, HID1], f32, space="PSUM", tag="mm")
                    nc.tensor.matmul(
                        out=zp[:w],
                        lhsT=xa[:, j * 128 : j * 128 + w],
                        rhs=w1a_t[:],
                        start=True,
                        stop=False,
                    )
                    nc.tensor.matmul(
                        out=zp[:w],
                        lhsT=xb[:, j * 128 : j * 128 + w],
                        rhs=w1b_t[:],
                        start=False,
                        stop=True,
                    )
                    zs = po.tile([128, HID1], bf16, tag="zs")
                    nc.vector.tensor_scalar_mul(
                        zs[:w], zp[:w], dinv_all_t[:w, nb_done + j : nb_done + j + 1]
                    )
                    nc.sync.dma_start(
                        out=z1_d[c0 + j * 128 : c0 + j * 128 + w], in_=zs[:w]
                    )
                nb_done += nb_cnt

            KPHASE = os.environ.get("KPHASE", "C")

            # ---- shared SpMM: gather + one-hot scatter matmul
            def spmm(src_d, m_rows, ht_all, qoff=0):
                qn = qoff
                for (cb0, cb1) in chunks:
                    ctA = sum(T[b][0] for b in range(cb0, cb1))
                    ctB = sum(T[b][1] for b in range(cb0, cb1))
                    offA, offB = toff[0][cb0], toff[1][cb0]
                    gA = pg.tile([128, ctA, 128], bf16, tag="gA")
                    gB = pg.tile([128, ctB, 128], bf16, tag="gB")
                    nc.gpsimd.dma_gather(
                        gA[:], src_d[0:HALF], idxA_t[:, offA * 8 : (offA + ctA) * 8],
                        ctA * 128, ctA * 128, 128, single_packet=False,
                        queue_num=qn % 4,
                    )
                    nc.gpsimd.dma_gather(
                        gB[:], src_d[HALF:N], idxB_t[:, offB * 8 : (offB + ctB) * 8],
                        ctB * 128, ctB * 128, 128, single_packet=False,
                        queue_num=(qn + 1) % 4,
                    )
                    qn += 2
                    for b in range(cb0, cb1):
                        acc_full = pp.tile([128, 128], f32, space="PSUM", tag="mm")
                        acc = acc_full[:m_rows]
                        n_mm = T[b][0] + T[b][1]
                        mi = 0
                        for h, g, dl_t, off0 in (
                            (0, gA, dlA_t, offA),
                            (1, gB, dlB_t, offB),
                        ):
                            tloc0 = toff[h][b] - off0
                            nt = T[b][h]
                            done = 0
                            while done < nt:
                                k = min(8, nt - done)
                                sel = psel.tile([128, k, 128], bf16, tag="sel")
                                nc.vector.tensor_tensor(
                                    out=sel[:],
                                    in0=dl_t[
                                        :, toff[h][b] + done : toff[h][b] + done + k
                                    ].to_broadcast([128, k, 128]),
                                    in1=iota_t[:, : k * 128],
                                    op=mybir.AluOpType.is_equal,
                                )
                                for q in range(k):
                                    nc.tensor.matmul(
                                        out=acc[:],
                                        lhsT=gA[:, tloc0 + done + q, 0:m_rows]
                                        if h == 0
                                        else gB[:, tloc0 + done + q, 0:m_rows],
                                        rhs=sel[:, q, :],
                                        start=(mi == 0),
                                        stop=(mi == n_mm - 1),
                                    )
                                    mi += 1
                                done += k
                        nc.vector.tensor_copy(
                            out=ht_all[:m_rows, b * 128 : (b + 1) * 128], in_=acc[:]
                        )

            # ---- phase B: SpMM1 -> h1^T ; z2' blocks -> z2l_d ; AllGather
            if KPHASE != "A":
                spmm(z1_d, 128, h1t_all)
            for b in range(NBLK) if KPHASE not in ("A",) else []:
                rows = min(128, NSH - b * 128)
                z2p = pp2.tile([128, HID2], f32, space="PSUM", tag="sm")
                nc.tensor.matmul(
                    out=z2p[:],
                    lhsT=h1t_all[:, b * 128 : (b + 1) * 128],
                    rhs=w2_t[:],
                    start=True,
                    stop=not has_bias,
                )
                if has_bias:
                    nc.tensor.matmul(
                        out=z2p[:],
                        lhsT=sqrow_t[0:1, b * 128 : (b + 1) * 128],
                        rhs=bz2_t[0:1, :],
                        start=False,
                        stop=True,
                    )
                z2s = po.tile([128, 128], bf16, tag="z2s")
                nc.vector.memset(z2s[:, HID2:128], 0.0)
                nc.vector.tensor_scalar_mul(
                    z2s[:, 0:HID2], z2p[:], ivcol_t[:, b : b + 1]
                )
                nc.sync.dma_start(
                    out=z2l_d[b * 128 : b * 128 + rows], in_=z2s[:rows]
                )
            if KPHASE in ("G", "C"):
                nc.gpsimd.collective_compute(
                    "AllGather",
                    mybir.AluOpType.bypass,
                    ins=[z2l_d[:]],
                    outs=[z2f_d[:]],
                    replica_groups=[core_ids],
                )

            # ---- phase C: SpMM2 -> h2^T ; mu/lv heads
            if KPHASE == "C":
                spmm(z2f_d, HID2, h2t_all)
            for b in range(NBLK) if KPHASE == "C" else []:
                rows = min(128, NSH - b * 128)
                for w_t, bias_t, out_e, tag in (
                    (wmu_t, "bmu", mu_e, "mup"),
                    (wlv_t, "blv", lv_e, "lvp"),
                ):
                    op = pp2.tile([128, OUT_D], f32, space="PSUM", tag="sm")
                    nc.tensor.matmul(
                        out=op[:],
                        lhsT=h2t_all[:, b * 128 : (b + 1) * 128],
                        rhs=w_t[:],
                        start=True,
                        stop=not has_bias,
                    )
                    if has_bias:
                        bt = bmu_t if bias_t == "bmu" else blv_t
                        nc.tensor.matmul(
                            out=op[:],
                            lhsT=sqrow_t[0:1, b * 128 : (b + 1) * 128],
                            rhs=bt[0:1, :],
                            start=False,
                            stop=True,
                        )
                    os_ = po.tile([128, OUT_D], f32, tag=tag + "s")
                    nc.vector.tensor_scalar_mul(os_[:], op[:], dvcol_t[:, b : b + 1])
                    nc.sync.dma_start(
                        out=out_e[b * 128 : b * 128 + rows], in_=os_[:rows]
                    )

    nc.compile()
    return nc


_CACHE = {}


def kernel(**inputs):
    in_maps, plan = _prep(**inputs)
    key = (tuple(tuple(t) for t in plan["T"]), plan["has_bias"])
    if key not in _CACHE:
        _CACHE[key] = _build(plan)
    nc = _CACHE[key]
    res = run_bass_kernel_spmd(nc, in_maps, list(range(NC_CORES)))
    mu = np.concatenate([res.results[c]["mu"] for c in range(NC_CORES)], axis=0)
    lv = np.concatenate([res.results[c]["lv"] for c in range(NC_CORES)], axis=0)
    return (mu, lv)
